# revision 70
# baseline (speedup 1.0000x reference)
"""Transformer-XL attention on 8 Trainium2 NeuronCores (Bass/Tile).

Sharding: 8 cores = 4 batches x 2 head-groups of 8 heads.
Each core computes its (batch, head-group) attention output projected through
its W_proj row-slice; host sums the two head-group partials per batch and adds
the bias terms (b_v @ W_proj + b_proj) once.

Structure (single continuous pipeline, no inter-phase barrier):
  1. Q proj -> R proj -> position pass (2a) for heads 0,1 -> K+V proj,
     with x streamed in 256-token chunks so DMA overlaps matmul.
  2. Per head h: content scores + exp + mul + AV, with the position pass
     for head h+1 interleaved (heads 0,1 pre-built during projections so
     the Activation engine is never idle during phase 1).
  3. Normalize (1/Z) per head-pair as soon as both heads of a column tile
     finish; final W_proj matmul at the end.
"""

import sys

for _p in ("/opt/trn_rl_repo",):
    if _p not in sys.path:
        sys.path.insert(0, _p)

from contextlib import ExitStack

import ml_dtypes
import numpy as np

import concourse.bacc as bacc
import concourse.bass as bass
import concourse.mybir as mybir
import concourse.tile as tile
from concourse.bass_utils import run_bass_kernel_spmd

CUR, FULL, BS, D = 1024, 2048, 4, 1024
HN, HD = 16, 64
PREV = FULL - CUR
SCALE = 1.0 / HD**0.5
HC = 8          # heads per core
CW = HC * HD    # 512 channel columns per core
BF = mybir.dt.bfloat16
F32 = mybir.dt.float32
EXP = mybir.ActivationFunctionType.Exp
BF_NP = ml_dtypes.bfloat16

_CACHE = {}


def _ap(t, off, dims):
    return bass.AP(tensor=t.tensor, offset=t.offset + off, ap=dims)


def _blk(d, rowlen, nblk):
    """DRAM [nblk*128, rowlen] viewed as [p, blk, col]."""
    return _ap(d, 0, [[rowlen, 128], [128 * rowlen, nblk], [1, rowlen]])


def build_program():
    nc = bacc.Bacc("TRN2", target_bir_lowering=False, debug=False)

    XcT = nc.dram_tensor("XcT", [D, CUR], BF, kind="ExternalInput").ap()
    XfT = nc.dram_tensor("XfT", [D, FULL], BF, kind="ExternalInput").ap()
    PosT = nc.dram_tensor("PosT", [D, FULL], BF, kind="ExternalInput").ap()
    Wq = nc.dram_tensor("Wq", [D, CW], BF, kind="ExternalInput").ap()
    Wk = nc.dram_tensor("Wk", [D, CW], BF, kind="ExternalInput").ap()
    Wv = nc.dram_tensor("Wv", [D, CW], BF, kind="ExternalInput").ap()
    Wpos = nc.dram_tensor("Wpos", [D, CW], BF, kind="ExternalInput").ap()
    Wproj = nc.dram_tensor("Wproj", [CW, D], BF, kind="ExternalInput").ap()
    qu_b_d = nc.dram_tensor("qu_b", [CW, 1], F32, kind="ExternalInput").ap()
    qv_b_d = nc.dram_tensor("qv_b", [CW, 1], F32, kind="ExternalInput").ap()
    k_b_d = nc.dram_tensor("k_b", [CW, 1], F32, kind="ExternalInput").ap()
    r_b_d = nc.dram_tensor("r_b", [CW, 1], F32, kind="ExternalInput").ap()
    out_d = nc.dram_tensor("out_part", [CUR, D], F32, kind="ExternalOutput").ap()
    z_dram = nc.dram_tensor("z_scratch", [HC, CUR], F32).ap()

    with tile.TileContext(nc) as tc, ExitStack() as ctx:
        persist = ctx.enter_context(tc.tile_pool(name="persist", bufs=1))
        ps_pool = ctx.enter_context(tc.tile_pool(name="ps", bufs=3, space="PSUM"))
        av_pool = ctx.enter_context(tc.tile_pool(name="avps", bufs=2, space="PSUM"))
        gpool = ctx.enter_context(tc.tile_pool(name="g", bufs=2))
        gspool = ctx.enter_context(tc.tile_pool(name="gs", bufs=3))
        gtpool = ctx.enter_context(tc.tile_pool(name="gt", bufs=2))

        QuT = persist.tile([128, 4 * CUR], BF, tag="QuT")
        QvT = persist.tile([128, 4 * CUR], BF, tag="QvT")
        KT = persist.tile([128, 4 * FULL], BF, tag="KT")
        RT = persist.tile([128, 4 * FULL], BF, tag="RT")
        Vp = persist.tile([128, 16 * 8 * 66], BF, tag="Vp")
        OT = persist.tile([128, 4 * CUR], BF, tag="OT")
        Zc = persist.tile([HC, CUR], F32, tag="Zc")

        biases = persist.tile([128, 16], F32, tag="biases")
        for bi, bd in enumerate((qu_b_d, qv_b_d, k_b_d, r_b_d)):
            nc.sync.dma_start(
                out=biases[:, bi * 4:(bi + 1) * 4],
                in_=_ap(bd, 0, [[1, 128], [128, 4]]),
            )

        # ones columns of V' (col 64 of each 66-wide head slot)
        nc.vector.memset(_ap(Vp, 64, [[16 * 8 * 66, 128], [8 * 66, 16], [66, 8], [1, 1]]), 1.0)

        # ---------------- position pass (2a) helpers ----------------
        pend_tr = []

        def do_2a_compute(h, qt, GT):
            ct = h // 2
            rb = (h % 2) * 64
            i0 = qt * 128
            m_lo = 896 - i0
            W = FULL - m_lo            # 1152 + i0
            Wj = i0 + 1152             # valid j width (multiple of 128)
            Re = W + 128
            expG = gpool.tile([128, Re], BF, tag="expG")
            off = 0
            while off < W:
                wc = min(1024, W - off)
                gps = ps_pool.tile([128, 1024], F32, tag="ps")
                sc = 0
                while sc < wc:
                    wn = min(512, wc - sc)
                    nc.tensor.matmul(
                        gps[:, sc:sc + wn],
                        QvT[rb:rb + 64, ct * CUR + i0: ct * CUR + i0 + 128],
                        RT[rb:rb + 64, ct * FULL + m_lo + off + sc:
                           ct * FULL + m_lo + off + sc + wn],
                        start=True, stop=True,
                    )
                    sc += wn
                nc.scalar.activation(expG[:, off:off + wc], gps[:, 0:wc], EXP, scale=SCALE)
                off += wc
            nc.vector.memset(expG[:, W:W + 128], 0.0)
            expGs = gspool.tile([128, 2176], BF, tag="expGs")
            # skew: expGs[p, j] = expG[p, 127 + j - p]
            nc.sync.dma_start(
                out=expGs[:, 0:Wj],
                in_=_ap(expG, 127, [[Re - 1, 128], [1, Wj]]),
            )
            pend_tr.append((qt, GT, expGs))

        def flush_2a_transpose(keep=0):
            while len(pend_tr) > keep:
                qt, GT, expGs = pend_tr.pop(0)
                Wj = qt * 128 + 1152
                nblk = qt + 9
                # transpose into GT[p, t, qt, f] = expGs[f, t*128 + p]
                nc.scalar.dma_start_transpose(
                    out=_ap(GT, qt * 2048, [[16 * 8 * 128, 128], [128, nblk], [1, 128]]),
                    in_=expGs[:, 0:Wj],
                )

        # ---------------- projections (streamed x chunks) ----------------
        def load_w(pool, w_dram, tag):
            w_sb = pool.tile([128, 8 * CW], BF, tag=tag)
            nc.sync.dma_start(out=w_sb.rearrange("p (kt c) -> p kt c", kt=8),
                              in_=_blk(w_dram, CW, 8))
            return w_sb

        def load_xchunk(pool, x_dram, n_free, c, ntok):
            xch = pool.tile([128, 8 * ntok], BF, tag="xch")
            nc.sync.dma_start(
                out=xch.rearrange("p (kt t) -> p kt t", kt=8),
                in_=_ap(x_dram, c * ntok,
                        [[n_free, 128], [128 * n_free, 8], [1, ntok]]),
            )
            return xch

        def proj_chunk(w_sb, xch, ntok, tokbase, n_free, out_sbs, bias_cols):
            for ct in range(4):
                ps = ps_pool.tile([128, 1024], F32, tag="ps")
                for kt in range(8):
                    nc.tensor.matmul(
                        ps[:, 0:ntok],
                        w_sb[:, kt * CW + ct * 128: kt * CW + ct * 128 + 128],
                        xch[:, kt * ntok: (kt + 1) * ntok],
                        start=(kt == 0), stop=(kt == 7),
                    )
                for o_sb, bcol in zip(out_sbs, bias_cols):
                    nc.vector.tensor_scalar(
                        o_sb[:, ct * n_free + tokbase: ct * n_free + tokbase + ntok],
                        ps[:, 0:ntok],
                        biases[:, bcol * 4 + ct: bcol * 4 + ct + 1],
                        None, mybir.AluOpType.add,
                    )

        NT = 256  # token chunk

        with tc.tile_pool(name="p1", bufs=2) as p1:
            # Q projection (XcT): 4 chunks
            wq_sb = load_w(p1, Wq, "w")
            for c in range(CUR // NT):
                xch = load_xchunk(p1, XcT, CUR, c, NT)
                proj_chunk(wq_sb, xch, NT, c * NT, CUR, [QuT, QvT], [0, 1])

            # R projection (PosT): 8 chunks
            wpos_sb = load_w(p1, Wpos, "w")
            for c in range(FULL // NT):
                xch = load_xchunk(p1, PosT, FULL, c, NT)
                proj_chunk(wpos_sb, xch, NT, c * NT, FULL, [RT], [3])

            # prefetch K/V weights before the position pass so SP issues them early
            wk_sb = load_w(p1, Wk, "w")
            wv_sb = load_w(p1, Wv, "w")

            # position pass for heads 0 and 1, interleaved with the K+V
            # projections (streamed per 256-token chunk of XfT) so the PE has
            # independent work while each 2a chain drains through Act/DMA
            gt0 = gtpool.tile([128, 16 * 8 * 128], BF, tag="GT")
            gt1 = gtpool.tile([128, 16 * 8 * 128], BF, tag="GT")
            twoa = [(0, qt) for qt in range(8)] + [(1, qt) for qt in range(8)]
            TPC = NT // 128  # V token-tiles per chunk
            for c in range(FULL // NT):
                for _ in range(2 * NT // 256):
                    h2, qt2 = twoa.pop(0)
                    do_2a_compute(h2, qt2, (gt0, gt1)[h2])
                flush_2a_transpose(keep=0)
                xch = load_xchunk(p1, XfT, FULL, c, NT)
                proj_chunk(wk_sb, xch, NT, c * NT, FULL, [KT], [2])
                for tt in range(TPC * c, TPC * c + TPC):
                    ps = ps_pool.tile([128, 1024], F32, tag="ps")
                    for kt in range(8):
                        nc.tensor.matmul(
                            ps[:, 0:512],
                            xch[:, kt * NT + (tt % TPC) * 128: kt * NT + (tt % TPC) * 128 + 128],
                            wv_sb[:, kt * CW: kt * CW + CW],
                            start=(kt == 0), stop=(kt == 7),
                        )
                    nc.vector.tensor_copy(
                        _ap(Vp, tt * 8 * 66, [[16 * 8 * 66, 128], [66, 8], [1, 64]]),
                        ps[:, 0:512].rearrange("p (h d) -> p h d", h=8),
                    )
            flush_2a_transpose(keep=0)

        # ---------------- attention + output ----------------
        with (
            tc.tile_pool(name="ec", bufs=4) as ecpool,
            tc.tile_pool(name="e", bufs=4) as epool,
            tc.tile_pool(name="st", bufs=2) as stpool,
            tc.tile_pool(name="zr", bufs=1) as zrpool,
            tc.tile_pool(name="ob", bufs=4) as obpool,
            tc.tile_pool(name="wp", bufs=1) as wppool,
        ):
            Wproj_sb = wppool.tile([128, 4 * D], BF, tag="Wproj")
            nc.gpsimd.dma_start(out=Wproj_sb.rearrange("p (ct d) -> p ct d", ct=4),
                                in_=_blk(Wproj, D, 4))

            def do_2b_scores(h, t):
                """content matmuls + exp; returns the expC tile."""
                ct = h // 2
                rb = (h % 2) * 64
                qt_min = max(0, t - 8)
                ioff = qt_min * 128
                w = CUR - ioff
                cps = ps_pool.tile([128, 1024], F32, tag="ps")
                sc = 0
                while sc < w:
                    wn = min(512, w - sc)
                    nc.tensor.matmul(
                        cps[:, sc:sc + wn],
                        KT[rb:rb + 64, ct * FULL + t * 128: ct * FULL + t * 128 + 128],
                        QuT[rb:rb + 64, ct * CUR + ioff + sc: ct * CUR + ioff + sc + wn],
                        start=True, stop=True,
                    )
                    sc += wn
                expC = ecpool.tile([128, 1024], BF, tag="expC")
                nc.scalar.activation(expC[:, 0:w], cps[:, 0:w], EXP, scale=SCALE)
                return expC

            def do_2b_mul(h, t, GT, expC):
                qt_min = max(0, t - 8)
                ioff = qt_min * 128
                w = CUR - ioff
                E = epool.tile([128, 1024], BF, tag="E")
                nqt = 8 - qt_min
                nc.vector.tensor_mul(
                    E[:, 0:w].rearrange("p (a f) -> p a f", f=128),
                    expC[:, 0:w].rearrange("p (a f) -> p a f", f=128),
                    _ap(GT, qt_min * 2048 + t * 128,
                        [[16 * 8 * 128, 128], [2048, nqt], [1, 128]]),
                )
                return E

            def do_2b_av(h, t, E, avs):
                ioff = max(0, t - 8) * 128
                for c in range(2):
                    lo = max(ioff, c * 512)
                    hi = (c + 1) * 512
                    if lo >= hi:
                        continue
                    last_t = 11 if c == 0 else 15
                    nc.tensor.matmul(
                        avs[c][:, lo - c * 512: hi - c * 512],
                        Vp[:, t * 8 * 66 + h * 66: t * 8 * 66 + h * 66 + 65],
                        E[:, lo - ioff: hi - ioff],
                        start=(t == 0), stop=(t == last_t),
                    )

            def do_evict(h, avs, fast=False):
                # the last head's evict is on the critical path into the final
                # projection; route its DMAs through the (now idle) HWDGE
                # queues instead of the slower SWDGE descriptor path
                eng = nc.sync if fast else nc.gpsimd
                ct = h // 2
                rb = (h % 2) * 64
                for c in range(2):
                    ost = stpool.tile([64, 512], BF, tag="ost")
                    nc.vector.tensor_copy(ost, avs[c][0:64, :])
                    eng.dma_start(
                        out=OT[rb:rb + 64, ct * CUR + c * 512: ct * CUR + c * 512 + 512],
                        in_=ost,
                    )
                    zst = stpool.tile([128, 512], F32, tag="zst")
                    nc.vector.tensor_copy(zst[64:65, :], avs[c][64:65, :])
                    eng.dma_start(
                        out=Zc[h:h + 1, c * 512: c * 512 + 512],
                        in_=zst[64:65, :],
                    )

            gts = {0: gt0, 1: gt1}
            for h in range(HC):
                build = h + 1 if (h >= 1 and h + 1 < HC) else None
                if build is not None:
                    gt_next = gtpool.tile([128, 16 * 8 * 128], BF, tag="GT")
                    gts[build] = gt_next
                av0 = av_pool.tile([65, 512], F32, tag="av")
                av1 = av_pool.tile([65, 512], F32, tag="av")
                avs = (av0, av1)
                for t in range(16):
                    expC = do_2b_scores(h, t)
                    if build is not None and t < 8:
                        do_2a_compute(build, t, gts[build])
                        flush_2a_transpose(keep=0)
                    E = do_2b_mul(h, t, gts[h], expC)
                    do_2b_av(h, t, E, avs)
                do_evict(h, avs, fast=(h == 7))
                gts.pop(h)

                # normalize column-tile ct = h//2 as soon as both heads done
                if h % 2 == 1:
                    ct = h // 2
                    # SWDGE mid-phase keeps HWDGE lane phasing constant; the
                    # last pair (ct3) is tail-critical, so use the idle HWDGE
                    # queues and split the two broadcast reads across engines
                    zw = nc.sync if h == 7 else nc.gpsimd
                    zw.dma_start(
                        out=_ap(z_dram, 2 * ct * CUR, [[CUR, 2], [1, CUR]]),
                        in_=Zc[2 * ct:2 * ct + 2, :],
                    )
                    zrep = zrpool.tile([128, CUR], F32, tag="zrep")
                    for a in range(2):
                        zr_eng = (nc.sync, nc.scalar)[a] if h == 7 else nc.gpsimd
                        zr_eng.dma_start(
                            out=zrep[a * 64:(a + 1) * 64, :],
                            in_=_ap(z_dram, (2 * ct + a) * CUR, [[0, 64], [1, CUR]]),
                        )
                    nc.vector.reciprocal(zrep, zrep)
                    nc.vector.tensor_mul(
                        OT[:, ct * CUR:(ct + 1) * CUR],
                        OT[:, ct * CUR:(ct + 1) * CUR],
                        zrep,
                    )

            # final output projection: rotate PSUM through both pools (4 tiles)
            # and alternate the PSUM->SBUF eviction between DVE and Act so the
            # 16-tile chain pipelines instead of serializing on one engine
            for it in range(8):
                for dc in range(2):
                    k = it * 2 + dc
                    pps = ps_pool.tile([128, 1024], F32, tag="ps")
                    for ct in range(4):
                        nc.tensor.matmul(
                            pps[:, 0:512],
                            OT[:, ct * CUR + it * 128: ct * CUR + it * 128 + 128],
                            Wproj_sb[:, ct * D + dc * 512: ct * D + dc * 512 + 512],
                            start=(ct == 0), stop=(ct == 3),
                        )
                    osb = obpool.tile([128, 512], F32, tag="osb")
                    if k % 2 == 0:
                        nc.vector.tensor_copy(osb, pps[:, 0:512])
                    else:
                        nc.scalar.activation(
                            osb, pps[:, 0:512],
                            mybir.ActivationFunctionType.Copy)
                    nc.sync.dma_start(
                        out=out_d[it * 128:(it + 1) * 128, dc * 512:(dc + 1) * 512],
                        in_=osb,
                    )

    nc.compile()
    return nc


def _prep_core_inputs(inputs, pos_embedding, full_input, u, v,
                      W_kv, b_kv, W_q, b_q, W_pos, b_pos, W_proj):
    """Host-side shard prep: returns list of 8 in_maps."""
    bf = BF_NP
    posT = np.ascontiguousarray(pos_embedding[:, 0, :].T).astype(bf)
    in_maps = []
    for c in range(8):
        b, hg = c // 2, c % 2
        s = slice(hg * CW, (hg + 1) * CW)
        hs = slice(hg * HC, (hg + 1) * HC)
        in_maps.append({
            "XcT": np.ascontiguousarray(inputs[:, b, :].T).astype(bf),
            "XfT": np.ascontiguousarray(full_input[:, b, :].T).astype(bf),
            "PosT": posT,
            "Wq": np.ascontiguousarray(W_q[:, s]).astype(bf),
            "Wk": np.ascontiguousarray(W_kv[:, :HN * HD][:, s]).astype(bf),
            "Wv": np.ascontiguousarray(W_kv[:, HN * HD:][:, s]).astype(bf),
            "Wpos": np.ascontiguousarray(W_pos[:, s]).astype(bf),
            "Wproj": np.ascontiguousarray(W_proj[s, :]).astype(bf),
            "qu_b": (b_q[s] + u[hs].reshape(-1)).astype(np.float32).reshape(CW, 1),
            "qv_b": (b_q[s] + v[hs].reshape(-1)).astype(np.float32).reshape(CW, 1),
            "k_b": b_kv[:HN * HD][s].astype(np.float32).reshape(CW, 1),
            "r_b": b_pos[s].astype(np.float32).reshape(CW, 1),
        })
    return in_maps


def kernel(inputs, pos_embedding, full_input, u, v, mask,
           W_kv, b_kv, W_q, b_q, W_pos, b_pos, W_proj, b_proj,
           _want_profile=False):
    inputs = np.asarray(inputs, np.float32)
    pos_embedding = np.asarray(pos_embedding, np.float32)
    full_input = np.asarray(full_input, np.float32)

    if "nc" not in _CACHE:
        _CACHE["nc"] = build_program()
    nc = _CACHE["nc"]

    in_maps = _prep_core_inputs(
        inputs, pos_embedding, full_input,
        np.asarray(u, np.float32), np.asarray(v, np.float32),
        np.asarray(W_kv, np.float32), np.asarray(b_kv, np.float32),
        np.asarray(W_q, np.float32), np.asarray(b_q, np.float32),
        np.asarray(W_pos, np.float32), np.asarray(b_pos, np.float32),
        np.asarray(W_proj, np.float32))

    res = run_bass_kernel_spmd(nc, in_maps, list(range(8)))

    b_v = np.asarray(b_kv, np.float32)[HN * HD:]
    beta = b_v @ np.asarray(W_proj, np.float32) + np.asarray(b_proj, np.float32)
    out = np.empty((CUR, BS, D), np.float32)
    for b in range(BS):
        out[:, b, :] = (res.results[2 * b]["out_part"]
                        + res.results[2 * b + 1]["out_part"] + beta)
    if _want_profile:
        return out, res
    return out
